# revision 7
# baseline (speedup 1.0000x reference)
"""DKVMN scatter_memory kernel for 8 Trainium2 NeuronCores.

Math: the reference scan only ever uses the (B, M, Dv) memory through
read @ Wf_r, so the whole recurrence collapses to a 32-dim linear
cumulative sum:

  S  = softmax(Eq @ Wa + ba)            (100 x 32)  per-vocab att rows
  cq = Eq @ Wf[:64] + bf                (100,)
  cv = Ev @ Wf[64:]                     (100,)
  w  = (2q + a) % 100
  pred[t,b] = cq[q[t,b]] + sum_{s<t} cv[w[s,b]] * <S[q[t,b]], S[q[s,b]]>

Per core (batch-sharded, Bs=128): the host precomputes a 120-row fp8
index encoding per token (pure index preprocessing; 0/1 exact in fp8):
rows 0:100 one-hot(q), rows 100:110 one-hot(w%10), rows 110:120
one-hot(w//10).  One 54-col matmul per batch element against a packed
table mcat = [S | cvt | ind | cq | pad] gathers the S-row, cq, and the
digit factors of cv[w] (cv[w] = sum_j 1{hi=j} * cv[10j+lo]).  The
cumsum over t is a strict-upper-triangular matmul.  Layout: t on
partitions, (b, m) on free dim.

v2 notes: host pre-transposes Eq/Ev (layout only) so the param phase
has no device transposes; all big input DMAs go through the sync
HWDGE ring in pass order so pass 0 can start as soon as its one-hot
chunk lands; per-pass PSUM->SBUF traffic is one fused scalar copy with
a 54-col (108B, 4B-aligned) stride so the DVE ops run in 2x mode;
the (b,m)->b segmented reduce runs on GpSimd.
"""
import functools
import numpy as np
import ml_dtypes

import concourse.bass as bass
import concourse.bacc as bacc
import concourse.mybir as mybir
from concourse import tile
from concourse.bass_utils import run_bass_kernel_spmd

T, B, M, DQ, DV, VOCAB = 128, 1024, 32, 64, 64, 100
NCORES = 8
BS = B // NCORES  # 128
N = T * BS        # tokens per core = 16384
R = 120           # one-hot rows: 100 q + 10 w-lo + 10 w-hi
GB = 32           # b per pass
NPASS = BS // GB  # 4
F32 = mybir.dt.float32
F16 = mybir.dt.float16
FP8 = mybir.dt.float8e4
AX = mybir.AxisListType
OP = mybir.AluOpType

# mcat column layout (53 used cols at stride 54):
#   0:32  S row      32:42 cvt (cv candidates given lo digit)
#   42:52 ind (1{hi=j})   52 cq   53 pad(0)
MC = 54

# packed-parameter column layout (f16 [128, PC])
_EQT, _EVT, _WAQ, _WFR, _BIA, _ONE, _US, _SKEL = (
    0, 100, 200, 233, 234, 268, 368, 496)
PC = _SKEL + MC  # 550


def _build():
    nc = bacc.Bacc("TRN2", num_devices=NCORES, debug=False, target_bir_lowering=False)
    d = {}
    d["pack"] = nc.dram_tensor("pack", [128, PC], F16, kind="ExternalInput").ap()
    d["ohall"] = nc.dram_tensor("ohall", [R, N], FP8, kind="ExternalInput").ap()
    preds = nc.dram_tensor("preds", [T, BS], F32, kind="ExternalOutput").ap()

    with tile.TileContext(nc) as tc:
        with (
            tc.tile_pool(name="sb", bufs=1) as sb,
            tc.tile_pool(name="oh", bufs=2) as ohp,
            tc.tile_pool(name="wk", bufs=2) as wk,
            tc.tile_pool(name="ps", bufs=3, space="PSUM") as ps,
        ):
            P = sb.tile([128, PC], F16)
            nc.sync.dma_start(P[:], d["pack"][:])
            # mcat skeleton: zeros + I10 at rows 110:120, cols 42:52
            mcat = sb.tile([R, MC], F16)
            nc.scalar.dma_start(mcat[:], d["pack"][0:R, _SKEL:_SKEL + MC])

            # one-hot chunks: pool rotation (bufs=2) gates chunk i+2 on
            # pass i's matmuls, so the DMAs pipeline under compute.
            oh_t = []
            for ci in range(NPASS):
                t_ = ohp.tile([R, GB * T], FP8, tag="oh", name=f"oh_{ci}")
                nc.sync.dma_start(t_[:], d["ohall"][:, ci * GB * T:(ci + 1) * GB * T])
                oh_t.append(t_)

            us_t = P[:, _US:_US + 128]

            # ---- parameter tables (no device transposes) ----
            # cv row first: it feeds the mcat spray DMA (longest dep chain)
            p_cvr = ps.tile([1, VOCAB], F32, tag="pP", bufs=2)
            nc.tensor.matmul(p_cvr[:], P[0:DV, _WFR:_WFR + 1],
                             P[0:DV, _EVT:_EVT + VOCAB], start=True, stop=True)
            cv_row = sb.tile([1, VOCAB], F16)
            nc.scalar.copy(cv_row[:], p_cvr[:])
            # Ev arrives row-permuted (perm(k) = 10(k%10) + k//10), so the cv
            # row comes out as cv_row[0, 10i+j] = cv[10j+i]; a plain [1,100]
            # -> [10,10] DMA spray then yields mcat[100+i, 32+j] = cv[10j+i].
            nc.scalar.dma_start(mcat[100:110, 32:42], cv_row[0:1, 0:VOCAB])

            # S and cq in one accumulation group: p_s = EqT.T@[Wa|Wfq] + [ba|bf]
            p_s = ps.tile([VOCAB, M + 1], F32, tag="pA")
            nc.tensor.matmul(p_s[:], P[0:DQ, _EQT:_EQT + VOCAB],
                             P[0:DQ, _WAQ:_WAQ + M + 1], start=True, stop=False)
            nc.tensor.matmul(p_s[:], P[0:1, _ONE:_ONE + VOCAB],
                             P[0:1, _BIA:_BIA + M + 1],
                             start=False, stop=True)
            nc.scalar.copy(mcat[0:VOCAB, 52:53], p_s[:, M:M + 1])
            mx_t = sb.tile([VOCAB, 1], F32)
            sm_t = sb.tile([VOCAB, 1], F32)
            se_t = sb.tile([VOCAB, M], F32)
            nc.vector.tensor_reduce(mx_t[:], p_s[:, 0:M], AX.X, OP.max)
            nc.vector.tensor_scalar_mul(mx_t[:], mx_t[:], -1.0)
            nc.scalar.activation(se_t[:], p_s[:, 0:M],
                                 mybir.ActivationFunctionType.Exp,
                                 bias=mx_t[:], scale=1.0)
            nc.vector.tensor_reduce(sm_t[:], se_t[:], AX.X, OP.add)
            nc.vector.reciprocal(sm_t[:], sm_t[:])
            nc.vector.tensor_scalar(out=mcat[0:VOCAB, 0:M], in0=se_t[:],
                                    scalar1=sm_t[:], scalar2=None, op0=OP.mult)

            # ---- main pipeline ----
            c_sb = sb.tile([128, BS], F32)

            for pi in range(NPASS):
                oh_g = oh_t[pi]
                pAs = []
                for half in range(2):
                    pA = ps.tile([128, 1024], F32, tag="pA", name=f"pA_{half}")
                    for k in range(16):
                        kb = half * 16 + k
                        nc.tensor.matmul(pA[:, k * 64:k * 64 + MC],
                                         oh_g[:, kb * T:(kb + 1) * T],
                                         mcat[:], start=True, stop=True)
                    pAs.append(pA)
                # one fused PSUM->SBUF copy per half: [S|cvt|ind|cq] at
                # stride 54 (108B, 4B-aligned)
                comp = wk.tile([128, 2 * 16 * MC], F16, tag="comp")
                for half in range(2):
                    nc.scalar.copy(
                        comp[:, half * 16 * MC:(half + 1) * 16 * MC].rearrange(
                            "p (k c) -> p k c", c=MC),
                        pAs[half][:].rearrange("p (k c) -> p k c", c=64)[:, :, 0:MC])
                c3 = comp[:].rearrange("p (k c) -> p k c", c=MC)
                # cq accumulation (col 52 of each 54-block)
                nc.scalar.copy(c_sb[:, pi * GB:(pi + 1) * GB], c3[:, :, 52:53])
                # cv[w] = sum_j cvt[j] * ind[j]
                cvp = wk.tile([128, GB * 10], F16, tag="cvp")
                cvw = wk.tile([128, GB], F16, tag="cvw")
                nc.vector.tensor_tensor(
                    cvp[:].rearrange("p (k c) -> p k c", c=10),
                    c3[:, :, 32:42], c3[:, :, 42:52], OP.mult)
                with nc.allow_low_precision(reason="10-term f16 dot of one-hot"):
                    nc.vector.tensor_reduce(
                        cvw[:], cvp[:].rearrange("p (k c) -> p k c", c=10),
                        AX.X, OP.add)
                # v = A * cv[w] (cv broadcast along m; stride-0 -> 1x mode)
                v_g = wk.tile([128, GB * M], F16, tag="v")
                a3 = c3[:, :, 0:M]
                cvb = cvw[:].rearrange("p (k c) -> p k c", c=1)
                a3b, cvb = bass.broadcast_tensor_aps(a3, cvb)
                nc.vector.tensor_tensor(
                    v_g[:].rearrange("p (k c) -> p k c", c=M), a3b, cvb, OP.mult)
                # exclusive cumsum over t (strict upper as lhsT), then C->f16
                c_g = wk.tile([128, GB * M], F16, tag="c")
                for half in range(2):
                    pP = ps.tile([128, 512], F32, tag="pP", name=f"pP_{half}", bufs=2)
                    nc.tensor.matmul(pP[:], us_t,
                                     v_g[:, half * 512:(half + 1) * 512],
                                     start=True, stop=True)
                    nc.scalar.copy(c_g[:, half * 512:(half + 1) * 512], pP[:])
                # pred contribution terms: A * C, then segmented reduce over m
                ap_p = wk.tile([128, GB * M], F16, tag="ap")
                nc.vector.tensor_tensor(
                    ap_p[:].rearrange("p (k c) -> p k c", c=M), a3,
                    c_g[:].rearrange("p (k c) -> p k c", c=M), OP.mult)
                o16 = wk.tile([128, GB], F16, tag="o16")
                osl = wk.tile([128, GB], F32, tag="osl")
                with nc.allow_low_precision(reason="32-term f16 dot, tol 2e-2"):
                    nc.vector.tensor_reduce(
                        o16[:], ap_p[:].rearrange("p (b m) -> p b m", m=M),
                        AX.X, OP.add)
                nc.vector.tensor_add(osl[:], o16[:], c_sb[:, pi * GB:(pi + 1) * GB])
                nc.sync.dma_start(preds[:, pi * GB:(pi + 1) * GB], osl[:])

    nc.compile()
    return nc


@functools.lru_cache(maxsize=1)
def _get_nc():
    return _build()


def _in_maps(questions, answers, Eq, Ev, Wa, ba, Wf, bf):
    questions = np.asarray(questions)
    answers = np.asarray(answers)
    w = (questions.astype(np.int64) * 2 + answers.astype(np.int64)) % VOCAB
    pack = np.zeros((128, PC), np.float16)
    pack[0:DQ, _EQT:_EQT + VOCAB] = np.asarray(Eq, np.float32).T
    # Ev rows permuted so the derived cv row is emitted in (i-major) order
    perm = np.array([10 * (k % 10) + k // 10 for k in range(VOCAB)])
    pack[0:DV, _EVT:_EVT + VOCAB] = np.asarray(Ev, np.float32)[perm].T
    wf = np.asarray(Wf, np.float32).reshape(DQ + DV)
    pack[0:DQ, _WAQ:_WAQ + M] = np.asarray(Wa, np.float32)
    pack[0:DQ, _WAQ + M] = wf[0:DQ]
    pack[0:DV, _WFR] = wf[DQ:DQ + DV]
    pack[0, _BIA:_BIA + M] = np.asarray(ba, np.float32).reshape(M)
    pack[0, _BIA + M] = np.asarray(bf, np.float32).reshape(())
    pack[0, _ONE:_ONE + VOCAB] = 1.0
    pack[:, _US:_US + 128] = np.triu(np.ones((128, 128), np.float16), k=1)
    # mcat skeleton: zeros except I10 at rows 110:120, cols 42:52
    skel = np.zeros((128, MC), np.float16)
    skel[110:120, 42:52] = np.eye(10, dtype=np.float16)
    pack[:, _SKEL:_SKEL + MC] = skel
    in_maps = []
    for c in range(NCORES):
        sl = slice(c * BS, (c + 1) * BS)
        qf = np.ascontiguousarray(questions[:, sl].T).ravel()
        wfl = np.ascontiguousarray(w[:, sl].T).ravel()
        oh = np.zeros((R, N), dtype=ml_dtypes.float8_e4m3)
        ar = np.arange(N)
        oh[qf, ar] = 1.0
        oh[100 + wfl % 10, ar] = 1.0
        oh[110 + wfl // 10, ar] = 1.0
        in_maps.append({"pack": pack, "ohall": oh})
    return in_maps


def kernel(questions, answers, Eq, Ev, Wa, ba, Wf, bf):
    nc = _get_nc()
    in_maps = _in_maps(questions, answers, Eq, Ev, Wa, ba, Wf, bf)
    res = run_bass_kernel_spmd(nc, in_maps, list(range(NCORES)))
    preds = np.concatenate([res.results[c]["preds"] for c in range(NCORES)], axis=1)
    return preds.astype(np.float32)
